# revision 38
# baseline (speedup 1.0000x reference)
"""DilatedAttention Trainium2 kernel (8 NeuronCores, SPMD).

Input  : q, k, v each (2, 24, 8192, 64) float32.
Output : same shape; per head-group windowed attention over dilated
         positions, non-dilated positions zero.

Sharding: 3 head groups x (b in 2, hg in 8) = 16 (b,head) pairs per
group. Core c takes pairs {2c, 2c+1} of every group -> 6 slices per
core, perfectly balanced, no cross-device communication.

The host-side shard step packs each slice's dilated rows into ONE
dense bf16 buffer per 32-seg chunk, laid out exactly as the kernel's
SBUF tile: [Q^T | K^T | V] concatenated on the free dim, Q^T/K^T
pre-transposed and duo-stacked (row h*64+dd = dd of seg 2u+h, col
u*mp+i), V duo-stacked with a host-prefilled ones column and padded
to 64 rows per half. Every device DMA is fully contiguous per
partition (~6KB descriptors), one SWDGE call per chunk. The unshard
step upcasts the dense bf16 output and scatters it back into the
full zero-filled f32 tensor.

Per-core kernel: per slice, process segments in chunks of 32 ("c4" =
4 quads of 8 segs = 16 duos). A duo packs 2 segments on partition
halves:
 - mm1 per half: lt[k,q] = K^T.T @ Q^T   (contraction d=64)
 - exp on ACT (PSUM->SBUF bf16, scale=1/sqrt(d); no max-subtraction
   needed: logits are O(5))
 - mm2 per half: [out_un | s] = e.T @ [V | 1]  (contraction k=m)
 - DVE reciprocal + one broadcast multiply per half into a bf16
   4-quad store tile; dense stores per half (sync/scalar HWDGE).

All PSUM tiles are full-bank sized: sub-bank PSUM tiles get packed at
non-bank-aligned offsets, and a matmul output that crosses a PSUM bank
boundary is fatal on hardware.
"""

import sys

if "/opt/trn_rl_repo" not in sys.path:
    sys.path.insert(0, "/opt/trn_rl_repo")

from contextlib import ExitStack

import numpy as np

import concourse.bass as bass  # noqa: F401
import concourse.mybir as mybir
import concourse.tile as tile
from concourse import bacc
from concourse.bass_utils import run_bass_kernel_spmd

B, H, S, D = 2, 24, 8192, 64
W_LIST = [64, 128, 256]
R_LIST = [1, 2, 4]
NG = 3
G = H // NG  # heads per group
N_CORES = 8
SCALE = 1.0 / (D**0.5)

# slice order per core: (group, pair_within_core)
SLICES = [(0, 0), (0, 1), (1, 0), (1, 1), (2, 0), (2, 1)]

# per-group geometry
GEO = []
for _g in range(NG):
    _w, _r = W_LIST[_g], R_LIST[_g]
    _off = _g * _r
    _m = len(range(_off, _w, _r))
    _n = S // _w
    GEO.append((_w, _r, _off, _m, _n))

F32 = mybir.dt.float32
BF16 = mybir.dt.bfloat16
BF16_NP = mybir.dt.np(BF16)

_PROGRAM = None
LAST_RESULT = None  # BassKernelResults of the most recent run (for test.py)


def build_slice(nc, tc, pools, ident, ph, oh, g):
    """Emit the program for one (b, head) slice of group g.

    ph: [NC4, 128, 32*mp + 16*65] packed [Q^T | K^T | V] bf16.
    oh: [NC4, 2, m, 16, 64] dense bf16 output.
    """
    qk, sb, ps, outp = pools
    _w, _r, _off, m, n = GEO[g]
    mp = m + (m & 1)  # even column pitch (PSUM bf16 needs 4B alignment)
    nc4 = n // 32
    # partition ranges of the two duo halves; one fused range when m == 64
    halves = [(0, 128)] if m == 64 else [(0, m), (64, 64 + m)]

    fw = 32 * mp + 1040
    for c4 in range(nc4):
        # ---- one packed load per chunk: [Q^T | K^T | V] concatenated
        # on the free dim, pre-transposed/duo-stacked by the host ----
        pk = qk.tile([128, fw], BF16, tag="pk")
        nc.gpsimd.dma_start(out=pk[:], in_=ph[c4])
        qt = pk[:, 0 : 16 * mp]
        kt = pk[:, 16 * mp : 32 * mp]
        vb = pk[:, 32 * mp :]  # [128, 1040] bf16, junk rows never read

        ost = None
        for tq in range(4):  # quad within the 32-seg chunk
            # ---- mm1: lt[k, q] per duo-half ----
            lt = ps.tile([128, 512], F32, tag="lt")  # full bank
            for j in range(4):
                du = 4 * tq + j
                qss = qt[:, du * mp : du * mp + m]
                kss = kt[:, du * mp : du * mp + m]
                nc.tensor.matmul(
                    lt[0:m, j * m : (j + 1) * m],
                    kss[0:64, :],
                    qss[0:64, :],
                    start=True,
                    stop=True,
                    tile_position=(0, 0),
                )
                nc.tensor.matmul(
                    lt[64 : 64 + m, j * m : (j + 1) * m],
                    kss[64:128, :],
                    qss[64:128, :],
                    start=True,
                    stop=True,
                    tile_position=(64, 64),
                )

            # ---- softmax numerator (per half: avoid unwritten parts) ----
            e = sb.tile([128, 4 * mp], BF16, tag="e")
            for p0, p1 in halves:
                if mp == m:
                    nc.scalar.activation(
                        e[p0:p1, :],
                        lt[p0:p1, 0 : 4 * m],
                        mybir.ActivationFunctionType.Exp,
                        scale=SCALE,
                    )
                else:
                    ev = e[p0:p1, :].rearrange("p (u x) -> p u x", x=mp)[
                        :, :, 0:m
                    ]
                    lv = lt[p0:p1, 0 : 4 * m].rearrange(
                        "p (u x) -> p u x", x=m
                    )
                    nc.scalar.activation(
                        ev, lv, mybir.ActivationFunctionType.Exp, scale=SCALE
                    )

            # ---- mm2: [out_un | s] = e.T @ [V | 1] per duo-half ----
            o_ps = ps.tile([128, 512], F32, tag="ops")  # full bank
            for j in range(4):
                jv = 4 * tq + j
                nc.tensor.matmul(
                    o_ps[0:m, j * 65 : (j + 1) * 65],
                    e[0:m, j * mp : j * mp + m],
                    vb[0:m, jv * 65 : (jv + 1) * 65],
                    start=True,
                    stop=True,
                    tile_position=(0, 0),
                )
                nc.tensor.matmul(
                    o_ps[64 : 64 + m, j * 65 : (j + 1) * 65],
                    e[64 : 64 + m, j * mp : j * mp + m],
                    vb[64 : 64 + m, jv * 65 : (jv + 1) * 65],
                    start=True,
                    stop=True,
                    tile_position=(64, 64),
                )

            # ---- normalize into the 4-quad dense store tile ----
            if tq == 0:
                ost = outp.tile([128, 1024], BF16, tag="ost")
            ob = tq * 256
            rcp = sb.tile([128, 4], F32, tag="rcp")
            opsv = o_ps[:, 0:260].rearrange("p (u e) -> p u e", e=65)
            ostv = ost[:, ob : ob + 256].rearrange("p (u e) -> p u e", e=64)
            for p0, p1 in halves:
                nc.vector.reciprocal(rcp[p0:p1, :], o_ps[p0:p1, 64:260:65])
                nc.vector.tensor_mul(
                    ostv[p0:p1],
                    opsv[p0:p1, :, 0:64],
                    rcp[p0:p1, :].unsqueeze(2).to_broadcast([p1 - p0, 4, 64]),
                )
            if tq == 3:
                nc.sync.dma_start(out=oh[c4, 0], in_=ost[0:m, :])
                nc.scalar.dma_start(out=oh[c4, 1], in_=ost[64 : 64 + m, :])


def make_pools(tc, stack):
    qk = stack.enter_context(tc.tile_pool(name="qk", bufs=12))
    sb = stack.enter_context(tc.tile_pool(name="sb", bufs=10))
    ps = stack.enter_context(tc.tile_pool(name="ps", bufs=3, space="PSUM"))
    outp = stack.enter_context(tc.tile_pool(name="outp", bufs=12))
    return qk, sb, ps, outp


def _build_program():
    nc = bacc.Bacc("TRN2", target_bir_lowering=False, debug=False)
    qs, ks, vs, os_ = [], [], [], []
    for sl, (g, _pair) in enumerate(SLICES):
        _w, _r, _off, m, n = GEO[g]
        nc4 = n // 32  # noqa: F841
        mp_ = m + (m & 1)
        qs.append(
            nc.dram_tensor(
                f"p{sl}",
                [nc4, 128, 32 * mp_ + 1040],
                BF16,
                kind="ExternalInput",
            ).ap()
        )
        os_.append(
            nc.dram_tensor(
                f"o{sl}", [nc4, 2, m, 16, 64], BF16, kind="ExternalOutput"
            ).ap()
        )

    with tile.TileContext(nc) as tc:
        with ExitStack() as stack:
            pools = make_pools(tc, stack)
            for sl, (g, _pair) in enumerate(SLICES):
                build_slice(nc, tc, pools, None, qs[sl], os_[sl], g)

    nc.finalize()
    return nc


def _get_program():
    global _PROGRAM
    if _PROGRAM is None:
        _PROGRAM = _build_program()
    return _PROGRAM


def _pack_slice(q2, k2, v2, g):
    """Pack one slice's Q^T | K^T | V into [NC4, 128, 32*mp + 1040].

    Q^T/K^T: row h*64+dd = dd of seg 2u+h, col u*mp+i. V: row h*64+i
    = dilated row i of seg 2u+h, col u*65+e with ones at e=64.
    """
    w, r, off, m, n = GEO[g]
    mp = m + (m & 1)
    nc4 = n // 32
    out = np.zeros((nc4, 128, 32 * mp + 1040), BF16_NP)
    for x, base in ((q2, 0), (k2, 16 * mp)):
        dense = x.reshape(n, w, D)[:, off :: r, :]
        blk = np.zeros((nc4, 128, 16, mp), BF16_NP)
        blk[:, :, :, 0:m] = (
            dense.reshape(nc4, 16, 2, m, D)
            .transpose(0, 2, 4, 1, 3)
            .reshape(nc4, 128, 16, m)
            .astype(BF16_NP)
        )
        out[:, :, base : base + 16 * mp] = blk.reshape(nc4, 128, 16 * mp)
    vdense = v2.reshape(n, w, D)[:, off :: r, :]
    vblk = np.zeros((nc4, 2, 64, 16, 65), BF16_NP)
    vblk[:, :, 0:m, :, 0:64] = (
        vdense.reshape(nc4, 16, 2, m, D)
        .transpose(0, 2, 3, 1, 4)
        .astype(BF16_NP)
    )
    vblk[:, :, :, :, 64] = 1.0
    out[:, :, 32 * mp :] = vblk.reshape(nc4, 128, 1040)
    return out


def _unpack_o(oh, g):
    """[NC4, 2, m, 16, 64] -> dense [n, m, 64]."""
    w, r, off, m, n = GEO[g]
    return (
        oh.reshape(n // 32, 2, m, 4, 4, D)
        .transpose(0, 3, 4, 1, 2, 5)
        .reshape(n, m, D)
    )


def kernel(q, k, v):
    global LAST_RESULT
    q = np.asarray(q, dtype=np.float32)
    k = np.asarray(k, dtype=np.float32)
    v = np.asarray(v, dtype=np.float32)
    assert q.shape == (B, H, S, D), q.shape

    nc = _get_program()

    # (b, head) pair p = b*G + hg within group g; core c owns p in {2c, 2c+1}
    in_maps = []
    for c in range(N_CORES):
        im = {}
        for sl, (g, j) in enumerate(SLICES):
            p = 2 * c + j
            b, hg = p // G, p % G
            head = g * G + hg
            im[f"p{sl}"] = _pack_slice(
                q[b, head], k[b, head], v[b, head], g
            )
        in_maps.append(im)

    LAST_RESULT = run_bass_kernel_spmd(nc, in_maps, core_ids=list(range(N_CORES)))

    out = np.zeros((B, H, S, D), np.float32)
    for c in range(N_CORES):
        for sl, (g, j) in enumerate(SLICES):
            p = 2 * c + j
            b, hg = p // G, p % G
            head = g * G + hg
            w, r, off, m, n = GEO[g]
            dense = _unpack_o(
                np.asarray(LAST_RESULT.results[c][f"o{sl}"]).astype(
                    np.float32
                ),
                g,
            )
            out[b, head].reshape(n, w, D)[:, off :: r, :] = dense
    return out
